# revision 19
# baseline (speedup 1.0000x reference)
"""DVAE GNN message-passing kernel for 8 Trainium2 NeuronCores.

Data parallel over batch B=2048 -> 256 graphs/core. Each core runs the full
20-step topological scan with all weights replicated.

Math (per sample b, step v in 0..19, Hfwd starts at 0):
  gated_u = sigmoid(Wg @ [H_u, e_u] + bg) * (Wm @ [H_u, e_u])
  Hin_v   = sum_u adj[b,u,v] * gated_u          (u >= v rows of Hfwd are 0,
            so gated_u there is a constant c_u)
  H_v     = GRUCell(x_v, Hin_v)
  mu,lv   = W1 @ H_19 + b1, W2 @ H_19 + b2

Device layout: batch-major activations [128b, feat]; matmuls run with the
activation (transposed via PE) as the stationary operand and weights moving,
so outputs land batch-major in PSUM. Biases and the vertex-id one-hot
contributions are folded into the matmuls via ones-rows / one-hot k-chunks.
The adj-weighted message sum runs as fused per-partition-scalar MACs
(scalar_tensor_tensor) split across DVE (batch tile 0) and GPSIMD (tile 1);
the constant part (u >= v) is a real matmul over the u axis seeding the
accumulator in PSUM.
"""

import sys
import numpy as np

for _p in ("/opt/trn_rl_repo",):
    if _p not in sys.path:
        sys.path.insert(0, _p)

B, MAXN, NVT, HS, NZ = 2048, 20, 26, 501, 56
HS2 = HS + 1                  # 502: fp32r needs even innermost free counts
NVT_EFF = NVT + MAXN          # 46
XDIM = NVT_EFF + 1            # 47
NCORES = 8
BS = B // NCORES              # 256 samples per core
G3 = 3 * HS                   # 1503
RZ = 2 * HS                   # 1002

# k-chunking of the augmented hidden axis (501 rows of H^T + ones row)
CH = [(0, 128), (128, 128), (256, 128), (384, 118)]  # covers 0..501 inclusive
# gated-side chunks: + vid one-hot rows appended (total 522 rows)
CHG = [(0, 128), (128, 128), (256, 128), (384, 128), (512, 10)]
CHH = [(0, 128), (128, 128), (256, 128), (384, 128), (512, 10)]  # H^T tile shapes

MM_DTYPE = "f32r"  # "f32r" (1 cyc/row, tf32-ish) | "f32" (4 cyc/row, exact)


def _pack_layout():
    """Column layout (fp32 elements) of the single packed static tensor.

    Returns (entries, ncols); entries: name -> (row0, nrows, col0, ncols).
    All matmul-consumed slices start at partition 0 or 64.
    """
    ents = {}
    col = 0

    def put(name, row0, nrows, ncols):
        nonlocal col
        ents[name] = (row0, nrows, col, ncols)
        col += ncols

    put("pk", 0, 84, MAXN * BS)          # rows 0:48 X^T+ones, 64:84 adjT masked
    for i, (o, s) in enumerate(CH):
        put(f"wrzh{i}", 0, s, 2 * HS2)
    for i, (o, s) in enumerate(CH):
        put(f"whn{i}", 0, s, HS2)
    put("wrzx", 0, XDIM + 1, 2 * HS2)
    put("wxnc", 0, 84, HS2)              # rows 0:48 W_in^T+bias, 64:84 C
    for i, (o, s) in enumerate(CHG):
        put(f"wg{i}", 0, s, HS2)
    for i, (o, s) in enumerate(CHG):
        put(f"wm{i}", 0, s, HS2)
    for i, (o, s) in enumerate(CH):
        put(f"w12{i}", 0, s, 2 * NZ)
    put("ident", 0, 128, 128)
    return ents, col


_PROG = None  # cached Bass program


def _build_program():
    import concourse.bass as bass
    import concourse.tile as tile
    from concourse import bacc, mybir

    f32 = mybir.dt.float32
    f32r = mybir.dt.float32r
    mdt = {"f32r": f32r, "f32": f32, "bf16": mybir.dt.bfloat16}[MM_DTYPE]
    AF = mybir.ActivationFunctionType
    OP = mybir.AluOpType

    nc = bacc.Bacc("TRN2", target_bir_lowering=False, debug=False)

    def din(name, shape, dt=None):
        return nc.dram_tensor(name, shape, dt or mdt, kind="ExternalInput").ap()

    ents, ncols = _pack_layout()
    d_wpack = din("wpack", [128, ncols])
    d_adjgb = din("adjgb", [BS, MAXN * MAXN], mybir.dt.bfloat16)
    d_ones1 = din("ones1", [1, 2 * 128])
    d_vr3 = din("vrows3", [MAXN, 11, 2 * 128])   # ones row + vid one-hot 0..9
    d_vr4 = din("vrows4", [MAXN, 10, 2 * 128])   # vid one-hot 10..19
    d_out = nc.dram_tensor("out", [BS, 2 * NZ], f32, kind="ExternalOutput").ap()

    def mm(out, lhsT, rhs, start, stop):
        nc.tensor.matmul(out, lhsT, rhs, start=start, stop=stop)

    with tile.TileContext(nc) as tc:
        with (
            tc.tile_pool(name="statics", bufs=1) as sp,
            tc.tile_pool(name="gstore", bufs=2 * (MAXN - 1)) as gp,
            tc.tile_pool(name="hint", bufs=2) as hip,
            tc.tile_pool(name="ht", bufs=2) as htp,
            tc.tile_pool(name="work1", bufs=1) as wp1,
            tc.tile_pool(name="work2", bufs=2) as wp2,
            tc.tile_pool(name="pp_rz", bufs=3, space="PSUM") as pp_rz,
            tc.tile_pool(name="pp_tps", bufs=3, space="PSUM") as pp_tps,
            tc.tile_pool(name="pp_hn", bufs=2, space="PSUM") as pp_hn,
        ):
            # ---- one packed static load: a single DMA -> a single wait sem ----
            WPACK = sp.tile([128, ncols], mdt, tag="wpack", name="wpack")
            nc.sync.dma_start(WPACK[:, :], d_wpack)

            def sl(name, dt=None):
                r0, nr, c0, ncl = ents[name]
                ap = WPACK[r0:r0 + nr, c0:c0 + ncl]
                return ap.bitcast(dt) if dt else ap

            PK = sl("pk")
            WRZH = [sl(f"wrzh{i}") for i in range(4)]
            WHN = [sl(f"whn{i}") for i in range(4)]
            WRZX = sl("wrzx")
            WXNC = sl("wxnc")
            WG = [sl(f"wg{i}") for i in range(5)]
            WM = [sl(f"wm{i}") for i in range(5)]
            W12 = [sl(f"w12{i}") for i in range(4)]
            IDN = sl("ident", f32)
            bf16 = mybir.dt.bfloat16
            ADJG = []
            for t in range(2):
                ab = sp.tile([128, MAXN * MAXN], bf16, tag=f"adjgb{t}",
                             name=f"adjgb{t}")
                nc.sync.dma_start(ab[:, :], d_adjgb[t * 128:(t + 1) * 128, :])
                ADJG.append(ab)

            # G storage: gated vectors per (vertex, batch-tile), bf16 so the
            # message chains run in the DVE 2x mode
            Gt = [[gp.tile([128, HS2], bf16, tag="g", name=f"g{_u}_{_t}")
                   for _t in range(2)] for _u in range(MAXN - 1)]

            ev = {0: nc.vector, 1: nc.gpsimd}  # per-batch-tile elementwise engine

            HT_final = None
            for v in range(MAXN):
                # ---- message input Hin_v, batch-major, per batch tile ----
                acc = []
                for t in range(2):
                    dps = pp_tps.tile([128, 512], f32, tag="tps", name=f"dps{v}_{t}")
                    # constant part: sum_{u>=v} adj[b,u,v] * C[u]
                    mm(dps[:, :HS2], PK[64:84, v * BS + t * 128:v * BS + (t + 1) * 128],
                       WXNC[64:84, :], start=True, stop=True)
                    a = wp2.tile([128, HS2], f32, tag=f"acc{t}", name=f"acc{v}_{t}")
                    if v == 0:
                        nc.scalar.copy(a[:, :], dps[:, :HS2])
                    else:
                        # bf16 fused MAC chain on DVE (2x mode in the middle)
                        ab = wp1.tile([128, HS2], bf16, tag=f"accb{t}",
                                      name=f"accb{v}_{t}")
                        cur = dps[:, :HS2]
                        for u in range(v):
                            dst = a[:, :] if u == v - 1 else ab[:, :]
                            nc.vector.scalar_tensor_tensor(
                                dst, Gt[u][t][:, :],
                                ADJG[t][:, u * MAXN + v:u * MAXN + v + 1],
                                cur, OP.mult, OP.add)
                            cur = dst
                    acc.append(a)

                # ---- transpose Hin -> Hinaug^T chunks ----
                HINT = [hip.tile([s, 2 * 128], mdt, tag=f"hint{i}", name=f"hint{v}_{i}")
                        for i, (o, s) in enumerate(CH)]
                for half in range(2):  # chunk pairs packed into one psum bank
                    tp = pp_tps.tile([128, 512], f32, tag="tps", name=f"tpi{v}_{half}")
                    for j in range(2):
                        i = half * 2 + j
                        o, s = CH[i]
                        w = min(s, HS - o)  # 128,128,128,117 real hidden rows
                        for t in range(2):
                            nc.tensor.transpose(
                                tp[:w, j * 256 + t * 128:j * 256 + (t + 1) * 128],
                                acc[t][:, o:o + w], IDN[:, :])
                    for j in range(2):
                        i = half * 2 + j
                        w = min(CH[i][1], HS - CH[i][0])
                        nc.scalar.copy(HINT[i][:w, :], tp[:w, j * 256:j * 256 + 256])
                nc.sync.dma_start(HINT[3][117:118, :], d_ones1)  # ones row -> b_hh

                # ---- gate matmuls ----
                rzp, hnp, inp = [], [], []
                for t in range(2):
                    xsl = PK[0:XDIM + 1, v * BS + t * 128:v * BS + (t + 1) * 128]
                    for j in range(2):  # r and z halves
                        ps = pp_rz.tile([128, 512], f32, tag="rz", name=f"rz{v}_{t}_{j}")
                        for i in range(4):
                            mm(ps[:, :HS2], HINT[i][:, t * 128:(t + 1) * 128],
                               WRZH[i][:, j * HS2:(j + 1) * HS2], start=(i == 0), stop=False)
                        mm(ps[:, :HS2], xsl, WRZX[:, j * HS2:(j + 1) * HS2],
                           start=False, stop=True)
                        rzp.append(ps)
                    hn = pp_hn.tile([128, 512], f32, tag="hn", name=f"hn{v}_{t}")
                    for i in range(4):
                        mm(hn[:, :HS2], HINT[i][:, t * 128:(t + 1) * 128], WHN[i][:, :],
                           start=(i == 0), stop=(i == 3))
                    hnp.append(hn)
                    ip = pp_tps.tile([128, 512], f32, tag="tps", name=f"in{v}_{t}")
                    mm(ip[:, :HS2], xsl, WXNC[0:XDIM + 1, :], start=True, stop=True)
                    inp.append(ip)

                # ---- GRU elementwise ----
                hb = []
                for t in range(2):
                    r = wp1.tile([128, HS2], f32, tag=f"r{t}", name=f"r{v}_{t}")
                    z = wp1.tile([128, HS2], f32, tag=f"z{t}", name=f"z{v}_{t}")
                    nc.scalar.activation(r[:, :HS], rzp[2 * t][:, :HS], AF.Sigmoid)
                    nc.scalar.activation(z[:, :HS], rzp[2 * t + 1][:, :HS], AF.Sigmoid)
                    # stage PSUM -> SBUF on ACT so the TT math can run on gpsimd
                    hnb = wp1.tile([128, HS], f32, tag=f"hnb{t}", name=f"hnb{v}_{t}")
                    nc.scalar.copy(hnb[:, :], hnp[t][:, :HS])
                    inb = wp1.tile([128, HS], f32, tag=f"inb{t}", name=f"inb{v}_{t}")
                    nc.scalar.copy(inb[:, :], inp[t][:, :HS])
                    tmp = wp1.tile([128, HS], f32, tag=f"tmp{t}", name=f"tmp{v}_{t}")
                    nc.gpsimd.tensor_tensor(tmp[:, :], r[:, :HS], hnb[:, :], OP.mult)
                    nc.gpsimd.tensor_tensor(tmp[:, :], tmp[:, :], inb[:, :], OP.add)
                    n = wp1.tile([128, HS], f32, tag=f"n{t}", name=f"n{v}_{t}")
                    nc.scalar.activation(n[:, :], tmp[:, :], AF.Tanh)
                    d = wp1.tile([128, HS], f32, tag=f"tmp{t}", name=f"d{v}_{t}")
                    nc.gpsimd.tensor_sub(d[:, :], acc[t][:, :HS], n[:, :])
                    h = wp2.tile([128, HS], f32, tag=f"h{t}", name=f"h{v}_{t}")
                    nc.gpsimd.tensor_tensor(h[:, :], d[:, :], z[:, :HS], OP.mult)
                    nc.gpsimd.tensor_tensor(h[:, :], h[:, :], n[:, :], OP.add)
                    hb.append(h)

                # ---- transpose H -> Haug^T chunks ----
                HT = [htp.tile([s, 2 * 128], mdt, tag=f"ht{i}", name=f"ht{v}_{i}")
                      for i, (o, s) in enumerate(CHH)]
                for half in range(2):
                    tp = pp_tps.tile([128, 512], f32, tag="tps", name=f"tph{v}_{half}")
                    for j in range(2):
                        i = half * 2 + j
                        o, s = CH[i]
                        w = min(s, HS - o)
                        for t in range(2):
                            nc.tensor.transpose(
                                tp[:w, j * 256 + t * 128:j * 256 + (t + 1) * 128],
                                hb[t][:, o:o + w], IDN[:, :])
                    for j in range(2):
                        i = half * 2 + j
                        w = min(CH[i][1], HS - CH[i][0])
                        nc.scalar.copy(HT[i][:w, :], tp[:w, j * 256:j * 256 + 256])
                # ones row (-> bg) + vid one-hot rows via DMA (partition-base
                # restrictions forbid engine writes at rows 117+)
                nc.sync.dma_start(HT[3][117:128, :], d_vr3[v, :, :])
                nc.sync.dma_start(HT[4][:, :], d_vr4[v, :, :])

                if v < MAXN - 1:
                    # ---- gated message for this vertex ----
                    for t in range(2):
                        zp = pp_rz.tile([128, 512], f32, tag="rz", name=f"zp{v}_{t}")
                        mp = pp_rz.tile([128, 512], f32, tag="rz", name=f"mp{v}_{t}")
                        for i in range(5):
                            mm(zp[:, :HS2], HT[i][:, t * 128:(t + 1) * 128], WG[i][:, :],
                               start=(i == 0), stop=(i == 4))
                        for i in range(5):
                            mm(mp[:, :HS2], HT[i][:, t * 128:(t + 1) * 128], WM[i][:, :],
                               start=(i == 0), stop=(i == 4))
                        sg = wp1.tile([128, HS2], f32, tag=f"r{t}", name=f"sg{v}_{t}")
                        nc.scalar.activation(sg[:, :], zp[:, :HS2], AF.Sigmoid)
                        mb = wp1.tile([128, HS2], f32, tag=f"z{t}", name=f"mb{v}_{t}")
                        nc.scalar.copy(mb[:, :], mp[:, :HS2])
                        nc.gpsimd.tensor_tensor(Gt[v][t][:, :], sg[:, :], mb[:, :], OP.mult)
                else:
                    HT_final = HT

            # ---- readout ----
            for t in range(2):
                op = pp_hn.tile([128, 512], f32, tag="hn", name=f"op{t}")
                for i in range(4):
                    ksl = (HT_final[i][:, t * 128:(t + 1) * 128] if i < 3
                           else HT_final[3][:118, t * 128:(t + 1) * 128])
                    mm(op[:, :2 * NZ], ksl,
                       W12[i][:, :], start=(i == 0), stop=(i == 3))
                ob = wp1.tile([128, 2 * NZ], f32, tag=f"ob{t}", name=f"ob{t}")
                nc.scalar.copy(ob[:, :], op[:, :2 * NZ])
                nc.sync.dma_start(d_out[t * 128:(t + 1) * 128, :], ob[:, :])

    nc.compile()
    return nc


def _host_prep(types, feats, adj, Wg, bg, Wm, W_ih, b_ih, W_hh, b_hh, W1, b1, W2, b2):
    """Build per-core input maps (numpy only)."""
    f = np.float32
    types = np.asarray(types).astype(np.int64)
    feats = np.asarray(feats, dtype=f)
    adj = np.asarray(adj, dtype=f)
    Wg, bg, Wm = np.asarray(Wg, f), np.asarray(bg, f), np.asarray(Wm, f)
    W_ih, b_ih = np.asarray(W_ih, f), np.asarray(b_ih, f)
    W_hh, b_hh = np.asarray(W_hh, f), np.asarray(b_hh, f)
    W1, b1 = np.asarray(W1, f), np.asarray(b1, f)
    W2, b2 = np.asarray(W2, f), np.asarray(b2, f)

    bsz = types.shape[0]
    ncore = NCORES
    bs = bsz // ncore

    # X^T with ones row: [48, MAXN*bs] per core
    X = np.zeros((bsz, MAXN, XDIM + 1), dtype=f)
    onehot = np.eye(NVT_EFF, dtype=f)[types.reshape(-1) % NVT_EFF]
    X[:, :, :NVT_EFF] = onehot.reshape(bsz, MAXN, NVT_EFF)
    X[:, :, NVT_EFF] = feats
    X[:, :, XDIM] = 1.0

    # constant gated vectors c_u for zero hidden state
    zg = 1.0 / (1.0 + np.exp(-(bg[None, :] + Wg[:, HS:].T)))   # [20, 501]
    C = (zg * Wm[:, HS:].T).astype(f)

    def aug(wT, brow):
        return np.concatenate([wT, brow[None, :]], axis=0).astype(f)

    def pad_rz(a):          # [s, 1002] -> [s, 1004] with per-gate 502 halves
        o = np.zeros((a.shape[0], 2 * HS2), dtype=f)
        o[:, :HS] = a[:, :HS]
        o[:, HS2:HS2 + HS] = a[:, HS:]
        return o

    def pad_h(a):           # [s, 501] -> [s, 502]
        o = np.zeros((a.shape[0], HS2), dtype=f)
        o[:, :HS] = a
        return o

    wrzh = pad_rz(aug(W_hh[:RZ].T, b_hh[:RZ]))
    whn = pad_h(aug(W_hh[RZ:].T, b_hh[RZ:]))
    wrzx = pad_rz(aug(W_ih[:RZ].T, b_ih[:RZ]))
    wxnc = np.zeros((84, HS2), dtype=f)
    wxnc[:XDIM + 1] = pad_h(aug(W_ih[RZ:].T, b_ih[RZ:]))
    wxnc[64:84] = pad_h(C)
    wg = pad_h(np.concatenate([Wg[:, :HS].T, bg[None, :], Wg[:, HS:].T], axis=0).astype(f))
    wm = pad_h(np.concatenate([Wm[:, :HS].T, np.zeros((1, HS), f), Wm[:, HS:].T],
                              axis=0).astype(f))
    w12 = np.concatenate([np.concatenate([W1.T, W2.T], axis=1),
                          np.concatenate([b1, b2])[None, :]], axis=0).astype(f)
    ident = np.eye(128, dtype=f)
    ones1 = np.ones((1, 256), dtype=f)
    vrows3 = np.zeros((MAXN, 11, 256), dtype=f)
    vrows3[:, 0, :] = 1.0                      # ones row -> bg
    for v in range(10):
        vrows3[v, 1 + v, :] = 1.0              # vid one-hot rows 0..9
    vrows4 = np.zeros((MAXN, 10, 256), dtype=f)
    for v in range(10, MAXN):
        vrows4[v, v - 10, :] = 1.0             # vid one-hot rows 10..19

    ents, ncols = _pack_layout()

    def place(pack, name, arr):
        r0, nr, c0, ncl = ents[name]
        assert arr.shape == (nr, ncl), (name, arr.shape, (nr, ncl))
        pack[r0:r0 + nr, c0:c0 + ncl] = arr

    umask = (np.arange(MAXN)[:, None] >= np.arange(MAXN)[None, :]).astype(f)

    in_maps = []
    for c in range(ncore):
        sl = slice(c * bs, (c + 1) * bs)
        Xc = X[sl]                                    # [bs, 20, 48]
        xt = Xc.transpose(2, 1, 0).reshape(XDIM + 1, MAXN * bs)
        adjc = adj[sl]                                # [bs, 20, 20]
        # adjT[u, v*bs+b] = adj[b,u,v], zeroed where u < v (only u>=v used)
        adjm = adjc.transpose(1, 2, 0) * umask[:, :, None]
        pk = np.zeros((84, MAXN * bs), dtype=f)
        pk[:XDIM + 1] = xt
        pk[64:84] = adjm.reshape(MAXN, MAXN * bs)

        pack = np.zeros((128, ncols), dtype=f)
        place(pack, "pk", pk)
        for i, (o, s) in enumerate(CH):
            place(pack, f"wrzh{i}", wrzh[o:o + s])
            place(pack, f"whn{i}", whn[o:o + s])
            place(pack, f"w12{i}", w12[o:o + s])
        place(pack, "wrzx", wrzx)
        place(pack, "wxnc", wxnc)
        for i, (o, s) in enumerate(CHG):
            place(pack, f"wg{i}", wg[o:o + s])
            place(pack, f"wm{i}", wm[o:o + s])
        place(pack, "ident", ident)
        import ml_dtypes
        adjgb = adjc.reshape(bs, MAXN * MAXN).astype(ml_dtypes.bfloat16)
        in_maps.append(dict(
            wpack=pack, adjgb=adjgb, ones1=ones1, vrows3=vrows3, vrows4=vrows4,
        ))
    return in_maps


def _get_prog():
    global _PROG
    if _PROG is None:
        _PROG = _build_program()
    return _PROG


def kernel(**inputs):
    from concourse.bass_utils import run_bass_kernel_spmd
    nc = _get_prog()
    in_maps = _host_prep(**inputs)
    res = run_bass_kernel_spmd(nc, in_maps, core_ids=list(range(NCORES)))
    out = np.concatenate([r["out"] for r in res.results], axis=0)
    mu = np.ascontiguousarray(out[:, :NZ])
    logvar = np.ascontiguousarray(out[:, NZ:])
    return mu, logvar
